# revision 13
# baseline (speedup 1.0000x reference)
"""BitLinearOptimized Trainium2 kernel — 8-core SPMD, self-contained.

kernel(**inputs) takes the FULL inputs (input [8192,4096] f32,
weight [4096,4096] f32 ternary, weight_scale [1] f32, bias [4096] f32)
and returns the FULL output [8192, 4096] f32.

Sharding: input row-sharded 8 ways (each core quantizes its rows),
weight sharded along out_features (each core group-sums its shard and
contributes w_sumT to AllGathers so every core holds all out features).

v4 (vs v2 baseline 439us, v3 465us):
- x loaded once, all 8 row tiles SBUF-resident, quantized in place;
  freed 16KB slots recycled as h0 matmul weight panels.
- Three collectives whose TRIGGER order is arranged so the CC stream
  never idles: AG1 (w_sumT half 0) triggers at ~28us and its flight
  hides inside the startup barrier window; the tiny absmax AllGather
  triggers as soon as the last x tile is reduced; AG2 (half 1) last.
- w group-sums on Vector, not GpSimd, so the CC cores stay quiet early
  (v3's early gpsimd work stretched the startup barrier 47->68us).
- w_sumT travels as fp8e4 (w_sum in [-4,4] is exact in e4m3): gather
  flights and panel loads halve; matmul runs fp8 lhsT x bf16 rhs.
- Transposes/g-stores issue on the scalar engine right behind the w0
  loads so they are not FIFO-blocked behind the bulk x backlog (the
  v3 mistake that pushed AG1's trigger to 106us).
- Quantize split across Vector/GpSimd (magic-round) and Scalar (round
  extraction); per-partition scale from a stride-0 broadcast read.
"""

import numpy as np

import concourse.bass as bass
from concourse import bacc
import concourse.mybir as mybir
import concourse.tile as tile

F32 = mybir.dt.float32
BF16 = mybir.dt.bfloat16
FP8 = mybir.dt.float8e4
MAGIC_C = float(np.float32(1.5 * 2**23))

# problem shape (hardcoded per contest contract)
N_FULL, IN_F, OUT_F, NCORES = 8192, 4096, 4096, 8


def build_bitlinear(N=N_FULL, IN=IN_F, OUT=OUT_F, ncores=NCORES):
    P = 128
    ROWS = N // ncores          # rows per core (1024)
    OCOLS = OUT // ncores       # out features per core (512)
    G = IN // 4                 # groups (1024)
    GH = G // 2                 # groups per AllGather half (512)
    RT = ROWS // P              # row tiles (8)
    GT = G // P                 # g tiles = matmul k tiles (8)
    OBT = OUT // P              # output o blocks (32)
    WT = OCOLS // P             # w shard row tiles (4)
    WCH = IN // 2               # w load chunk = one g-half of in features
    NN = 2                      # moving-dim split
    NCH = ROWS // NN            # 512
    assert ROWS % P == 0 and G % P == 0 and OCOLS % P == 0

    core_ids = list(range(ncores))

    nc = bacc.Bacc(num_devices=ncores)

    x_d = nc.declare_dram_parameter("x_loc", [ROWS, IN], F32, isOutput=False)
    w_d = nc.declare_dram_parameter("w_loc", [OCOLS, IN], F32, isOutput=False)
    ws_d = nc.declare_dram_parameter("wscale", [1, 1], F32, isOutput=False)
    bias_d = nc.declare_dram_parameter("bias", [OUT], F32, isOutput=False)
    outT_d = nc.declare_dram_parameter("outT", [OUT, ROWS], F32, isOutput=True)

    # collective bounce buffers
    mx_in = nc.dram_tensor("mx_in", [1, P], BF16)
    mx_out = nc.dram_tensor("mx_out", [ncores, P], BF16, addr_space="Shared")
    g1_in = nc.dram_tensor("g1_in", [GH, OCOLS], FP8)
    g1_out = nc.dram_tensor("g1_out", [ncores * GH, OCOLS], FP8,
                            addr_space="Shared")
    g2_in = nc.dram_tensor("g2_in", [GH, OCOLS], FP8)
    g2_out = nc.dram_tensor("g2_out", [ncores * GH, OCOLS], FP8,
                            addr_space="Shared")

    with tile.TileContext(nc) as tc:
        with (
            tc.tile_pool(name="xa", bufs=8) as xap,       # resident x, recycled as w panels
            tc.tile_pool(name="scr", bufs=3) as scrp,     # w loads / qt / h1 panels
            tc.tile_pool(name="qab", bufs=2) as qabp,
            tc.tile_pool(name="xs", bufs=3) as xsp,
            tc.tile_pool(name="xsT", bufs=1) as xsTp,
            tc.tile_pool(name="wab", bufs=4) as wabp,
            tc.tile_pool(name="wsh", bufs=4) as wshp,
            tc.tile_pool(name="w3T", bufs=2) as w3Tp,
            tc.tile_pool(name="w3T8", bufs=2) as w3T8p,
            tc.tile_pool(name="outp", bufs=3) as outp,
            tc.tile_pool(name="cst", bufs=1) as cst,
            tc.tile_pool(name="ps", bufs=4, space="PSUM") as psp,
        ):

            # ---------------- phase A loads ----------------------------------
            #  sync ring:   w0.wt0 w0.wt1 x0 x2 x4 x6 w1.wt0 w1.wt1
            #  scalar ring: w0.wt2 w0.wt3 [T-h0 + g1 stores] x1 x3 x5 x7
            #               w1.wt2 w1.wt3 [T-h1 + g2 stores]
            wl = {}
            xa = [None] * RT

            def load_w(eng, h, wt):
                t = scrp.tile([P, WCH], F32, tag="scr", name=f"wl{h}_{wt}")
                eng.dma_start(out=t[:], in_=w_d[wt * P:(wt + 1) * P,
                                               h * WCH:(h + 1) * WCH])
                wl[(h, wt)] = t

            def load_x(eng, rt):
                t = xap.tile([P, IN], F32, tag="xa", name=f"xa{rt}")
                eng.dma_start(out=t[:], in_=x_d[rt * P:(rt + 1) * P, :])
                xa[rt] = t

            # w sums on Vector; transpose on scalar (xbar needs bf16),
            # fp8 convert on Vector, store on scalar.
            wsh_t = {}

            def w_sum(h, wt):
                wlh = wl[(h, wt)]
                w3 = wlh[:].rearrange("p (g f) -> p g f", f=4)
                wa = wabp.tile([P, GH], BF16, tag="wab", name=f"wa{h}_{wt}")
                wb = wabp.tile([P, GH], BF16, tag="wab", name=f"wb{h}_{wt}")
                nc.vector.tensor_tensor(out=wa[:], in0=w3[:, :, 0], in1=w3[:, :, 1],
                                        op=mybir.AluOpType.add)
                nc.vector.tensor_tensor(out=wb[:], in0=w3[:, :, 2], in1=w3[:, :, 3],
                                        op=mybir.AluOpType.add)
                wsh = wshp.tile([P, GH], BF16, tag="wsh", name=f"wsh{h}_{wt}")
                nc.vector.tensor_tensor(out=wsh[:], in0=wa[:], in1=wb[:],
                                        op=mybir.AluOpType.add)
                wsh_t[(h, wt)] = wsh

            def w_tx(h, wt, gdst):
                # [128 o, 512 g] -(xbar)-> [128 gp, 4, 128 o] bf16 -> fp8 -> DRAM
                w3T = w3Tp.tile([P, GH // P, P], BF16, tag="w3T",
                                name=f"w3T{h}_{wt}")
                nc.scalar.dma_start_transpose(w3T[:], wsh_t[(h, wt)][:])
                w38 = w3T8p.tile([P, GH // P, P], FP8, tag="w3T8",
                                 name=f"w38{h}_{wt}")
                nc.vector.tensor_copy(out=w38[:], in_=w3T[:])
                nc.scalar.dma_start(
                    out=gdst[0:GH, wt * P:(wt + 1) * P]
                        .rearrange("(a p) o -> p a o", p=P),
                    in_=w38[:])

            load_x(nc.sync, 0)
            load_x(nc.scalar, 1)
            load_x(nc.sync, 2)
            load_x(nc.scalar, 3)
            load_x(nc.sync, 4)
            load_x(nc.scalar, 5)
            load_x(nc.sync, 6)
            load_x(nc.scalar, 7)
            load_w(nc.sync, 0, 0)
            load_w(nc.scalar, 0, 2)
            load_w(nc.sync, 0, 1)
            load_w(nc.scalar, 0, 3)
            load_w(nc.sync, 1, 0)
            load_w(nc.scalar, 1, 2)
            load_w(nc.sync, 1, 1)
            load_w(nc.scalar, 1, 3)

            # ---------------- absmax -> tiny AllGather -----------------------
            mxcol = cst.tile([P, RT], F32, tag="mxcol")
            for rt in [0, 1, 2, 3, 4, 5, 6, 7]:
                nc.vector.tensor_reduce(out=mxcol[:, rt:rt + 1], in_=xa[rt][:],
                                        axis=mybir.AxisListType.X,
                                        op=mybir.AluOpType.max,
                                        apply_absolute_value=True)
            mx1 = cst.tile([P, 1], F32, tag="mx1")
            nc.vector.tensor_reduce(out=mx1[:], in_=mxcol[:],
                                    axis=mybir.AxisListType.X,
                                    op=mybir.AluOpType.max)
            mxbf = cst.tile([P, 1], BF16, tag="mxbf")
            nc.vector.tensor_copy(out=mxbf[:], in_=mx1[:])
            nc.gpsimd.dma_start(out=mx_in[0:1, :].rearrange("a p -> p a"),
                                in_=mxbf[:])
            nc.gpsimd.collective_compute(
                "AllGather", mybir.AluOpType.bypass,
                replica_groups=[core_ids],
                ins=[mx_in[:]], outs=[mx_out[:]],
            )

            # ---------------- w sums + gathers -------------------------------
            for wt in (0, 2, 1, 3):
                w_sum(0, wt)
            for wt in range(WT):
                w_tx(0, wt, g1_in)
            nc.gpsimd.collective_compute(
                "AllGather", mybir.AluOpType.bypass,
                replica_groups=[core_ids],
                ins=[g1_in[:]], outs=[g1_out[:]],
            )
            ws_bc = cst.tile([P, 1], F32, tag="ws_bc")
            nc.scalar.dma_start(out=ws_bc[:],
                                in_=bass.AP(ws_d, 0, [[0, P], [1, 1]]))
            bias_sb = cst.tile([P, OBT], F32, tag="bias_sb")
            nc.scalar.dma_start(out=bias_sb[:],
                                in_=bias_d[:].rearrange("(b p) -> p b", p=P))
            for wt in (2, 0, 3, 1):
                w_sum(1, wt)
            for wt in range(WT):
                w_tx(1, wt, g2_in)
            # global scale: broadcast-read the gathered maxes on the (now
            # idle) scalar ring, then AG2 flies behind AG1 on the CC stream
            mxg = cst.tile([P, ncores * P], BF16, tag="mxg")
            nc.scalar.dma_start(
                out=mxg[:].rearrange("p (c o) -> p c o", c=ncores),
                in_=bass.AP(mx_out, 0, [[0, P], [P, ncores], [1, P]]))
            nc.gpsimd.collective_compute(
                "AllGather", mybir.AluOpType.bypass,
                replica_groups=[core_ids],
                ins=[g2_in[:]], outs=[g2_out[:]],
            )
            mx2 = cst.tile([P, 1], F32, tag="mx2")
            nc.vector.tensor_reduce(out=mx2[:], in_=mxg[:],
                                    axis=mybir.AxisListType.X,
                                    op=mybir.AluOpType.max)
            # recip = 127/max ; asc = max/127 ; sc = ws*asc*0.25
            recip = cst.tile([P, 1], F32, tag="recip")
            nc.vector.reciprocal(out=recip[:], in_=mx2[:])
            nc.vector.tensor_scalar(out=recip[:], in0=recip[:], scalar1=127.0,
                                    scalar2=None, op0=mybir.AluOpType.mult)
            asc = cst.tile([P, 1], F32, tag="asc")
            nc.vector.tensor_scalar(out=asc[:], in0=mx2[:],
                                    scalar1=float(np.float32(1.0 / 127.0)),
                                    scalar2=None, op0=mybir.AluOpType.mult)
            sc_bc = cst.tile([P, 1], F32, tag="sc_bc")
            nc.vector.tensor_tensor(out=sc_bc[:], in0=ws_bc[:], in1=asc[:],
                                    op=mybir.AluOpType.mult)
            nc.vector.tensor_scalar(out=sc_bc[:], in0=sc_bc[:], scalar1=0.25,
                                    scalar2=None, op0=mybir.AluOpType.mult)

            # ---------------- quantize + group-sum + transpose ---------------
            xsT3 = xsTp.tile([P, GT, ROWS], BF16, tag="xsT3")
            qts = {}
            xs_t = {}
            panels = {}

            def ts1(rt):
                eng = nc.vector if rt % 2 == 0 else nc.gpsimd
                eng.tensor_scalar(out=xa[rt][:], in0=xa[rt][:],
                                  scalar1=recip[:], scalar2=MAGIC_C,
                                  op0=mybir.AluOpType.mult,
                                  op1=mybir.AluOpType.add)

            def ext(rt):
                qt = scrp.tile([P, IN], BF16, tag="scr", name=f"qt{rt}")
                nc.scalar.activation(out=qt[:], in_=xa[rt][:],
                                     func=mybir.ActivationFunctionType.Copy,
                                     bias=-MAGIC_C, scale=1.0)
                qts[rt] = qt

            def adds(rt):
                eng = nc.vector if rt % 2 == 0 else nc.gpsimd
                q3 = qts[rt][:].rearrange("p (g f) -> p g f", f=4)
                qa = qabp.tile([P, G], BF16, tag="qab", name=f"qa{rt}")
                qb = qabp.tile([P, G], BF16, tag="qab", name=f"qb{rt}")
                eng.tensor_tensor(out=qa[:], in0=q3[:, :, 0], in1=q3[:, :, 1],
                                  op=mybir.AluOpType.add)
                eng.tensor_tensor(out=qb[:], in0=q3[:, :, 2], in1=q3[:, :, 3],
                                  op=mybir.AluOpType.add)
                xs = xsp.tile([P, G], BF16, tag="xs", name=f"xs{rt}")
                eng.tensor_tensor(out=xs[:], in0=qa[:], in1=qb[:],
                                  op=mybir.AluOpType.add)
                xs_t[rt] = xs

            def tx(rt):
                nc.sync.dma_start_transpose(
                    xsT3[:, :, rt * P:(rt + 1) * P], xs_t[rt][:])

            def panel(c, h):
                # h0 panels recycle the xa slots (gated by ext(c)); h1 panels
                # recycle the scr slots (gated by the late qt adds).
                if h == 0:
                    t = xap.tile([P, GH // P, OCOLS], FP8, tag="xa",
                                 name=f"wst{c}_{h}")
                    eng = nc.scalar if c < 4 else nc.sync
                else:
                    t = scrp.tile([P, GH // P, OCOLS], FP8, tag="scr",
                                  name=f"wst{c}_{h}")
                    eng = nc.sync if c % 2 == 0 else nc.scalar
                src = g1_out if h == 0 else g2_out
                eng.dma_start(
                    out=t[:],
                    in_=src[c * GH:(c + 1) * GH, :]
                        .rearrange("(a p) o -> p a o", p=P))
                panels[(c, h)] = t

            for rt in (0, 2, 4, 6):
                ts1(rt)
            for rt in (1, 3, 5, 7):
                ts1(rt)
            ext(0)
            ext(1)
            panel(0, 0)
            ext(2)
            panel(1, 0)
            ext(3)
            panel(2, 0)
            ext(4)
            panel(3, 0)
            adds(0)
            adds(1)
            tx(0)
            ext(5)
            adds(2)
            adds(3)
            tx(1)
            tx(2)
            ext(6)
            adds(4)
            tx(3)
            panel(4, 0)
            ext(7)
            adds(5)
            adds(6)
            tx(4)
            panel(5, 0)
            tx(5)
            panel(6, 0)
            adds(7)
            tx(6)
            panel(7, 0)
            tx(7)
            for c in range(8):
                panel(c, 1)

            # ---------------- matmul + epilogue ------------------------------
            # per source core c: nn0 k<4, nn0 k>=4, nn1 k<4, nn1 k>=4 so the
            # stream meets each gate (panels h0, AG2 panels, late xsT3
            # columns) in the order they become ready.
            epi_cnt = 0
            for c in range(ncores):
                p1 = panels[(c, 0)]
                p2 = panels[(c, 1)]
                pss = {}
                for oc in range(WT):
                    ob = c * WT + oc
                    pss[oc] = psp.tile([P, ROWS], F32, tag="ps", name=f"ps{ob}")
                for nn in range(NN):
                    for half, ph in ((0, p1), (1, p2)):
                        for oc in range(WT):
                            for kk in range(GT // 2):
                                k = half * 4 + kk
                                nc.tensor.matmul(
                                    pss[oc][:, nn * NCH:(nn + 1) * NCH],
                                    lhsT=ph[:, kk, oc * P:(oc + 1) * P],
                                    rhs=xsT3[:, k, nn * NCH:(nn + 1) * NCH],
                                    start=(k == 0), stop=(k == GT - 1))
                for oc in range(WT):
                    ob = c * WT + oc
                    ot = outp.tile([P, ROWS], F32, tag="outp", name=f"ot{ob}")
                    if epi_cnt % 2 == 0:
                        nc.vector.tensor_scalar(out=ot[:], in0=pss[oc][:],
                                                scalar1=sc_bc[:],
                                                scalar2=bias_sb[:, ob:ob + 1],
                                                op0=mybir.AluOpType.mult,
                                                op1=mybir.AluOpType.add)
                    else:
                        nc.scalar.activation(
                            out=ot[:], in_=pss[oc][:],
                            func=mybir.ActivationFunctionType.Identity,
                            scale=sc_bc[:],
                            bias=bias_sb[:, ob:ob + 1])
                    epi_cnt += 1
                    seng = nc.sync if ob % 2 == 0 else nc.scalar
                    seng.dma_start(
                        out=outT_d[ob * P:(ob + 1) * P, :], in_=ot[:])

    return nc


def make_in_maps(inputs, ncores=NCORES):
    x = np.ascontiguousarray(np.asarray(inputs["input"], dtype=np.float32))
    w = np.ascontiguousarray(np.asarray(inputs["weight"], dtype=np.float32))
    ws = np.asarray(inputs["weight_scale"], dtype=np.float32).reshape(1, 1)
    b = np.ascontiguousarray(np.asarray(inputs["bias"], dtype=np.float32))
    N = x.shape[0]
    OUT = w.shape[0]
    ROWS = N // ncores
    OCOLS = OUT // ncores
    return [
        {
            "x_loc": x[c * ROWS:(c + 1) * ROWS],
            "w_loc": w[c * OCOLS:(c + 1) * OCOLS],
            "wscale": ws,
            "bias": b,
        }
        for c in range(ncores)
    ]


def assemble_output(results):
    return np.ascontiguousarray(
        np.concatenate([np.asarray(r["outT"]).T for r in results], axis=0))


_NC_CACHE = {}


def _get_nc():
    key = (N_FULL, IN_F, OUT_F, NCORES)
    if key not in _NC_CACHE:
        nc = build_bitlinear(*key)
        if not nc.is_finalized():
            nc.finalize()
        _NC_CACHE[key] = nc
    return _NC_CACHE[key]


def run_on_hw(inputs, trace=False):
    from concourse.bass_utils import run_bass_kernel_spmd
    nc = _get_nc()
    in_maps = make_in_maps(inputs)
    res = run_bass_kernel_spmd(nc, in_maps, list(range(NCORES)), trace=trace)
    return assemble_output(res.results), res


def kernel(**inputs) -> np.ndarray:
    out, _ = run_on_hw(inputs, trace=False)
    return out


# revision 15
# speedup vs baseline: 1.0216x; 1.0216x over previous
"""BitLinearOptimized Trainium2 kernel — 8-core SPMD, self-contained.

kernel(**inputs) takes the FULL inputs (input [8192,4096] f32,
weight [4096,4096] f32 ternary, weight_scale [1] f32, bias [4096] f32)
and returns the FULL output [8192, 4096] f32.

Sharding: input row-sharded 8 ways (each core quantizes its rows),
weight sharded along out_features (each core group-sums its shard and
contributes w_sumT to AllGathers so every core holds all out features).

v4 (vs v2 baseline 439us, v3 465us):
- x loaded once, all 8 row tiles SBUF-resident, quantized in place;
  freed 16KB slots recycled as h0 matmul weight panels.
- Three collectives whose TRIGGER order is arranged so the CC stream
  never idles: AG1 (w_sumT half 0) triggers at ~28us and its flight
  hides inside the startup barrier window; the tiny absmax AllGather
  triggers as soon as the last x tile is reduced; AG2 (half 1) last.
- w group-sums on Vector, not GpSimd, so the CC cores stay quiet early
  (v3's early gpsimd work stretched the startup barrier 47->68us).
- w_sumT travels as fp8e4 (w_sum in [-4,4] is exact in e4m3): gather
  flights and panel loads halve; matmul runs fp8 lhsT x bf16 rhs.
- Transposes/g-stores issue on the scalar engine right behind the w0
  loads so they are not FIFO-blocked behind the bulk x backlog (the
  v3 mistake that pushed AG1's trigger to 106us).
- Quantize split across Vector/GpSimd (magic-round) and Scalar (round
  extraction); per-partition scale from a stride-0 broadcast read.
"""

import numpy as np

import concourse.bass as bass
from concourse import bacc
import concourse.mybir as mybir
import concourse.tile as tile

F32 = mybir.dt.float32
BF16 = mybir.dt.bfloat16
FP8 = mybir.dt.float8e4
MAGIC_C = float(np.float32(1.5 * 2**23))

# problem shape (hardcoded per contest contract)
N_FULL, IN_F, OUT_F, NCORES = 8192, 4096, 4096, 8


def build_bitlinear(N=N_FULL, IN=IN_F, OUT=OUT_F, ncores=NCORES):
    P = 128
    ROWS = N // ncores          # rows per core (1024)
    OCOLS = OUT // ncores       # out features per core (512)
    G = IN // 4                 # groups (1024)
    GH = G // 2                 # groups per AllGather half (512)
    RT = ROWS // P              # row tiles (8)
    GT = G // P                 # g tiles = matmul k tiles (8)
    OBT = OUT // P              # output o blocks (32)
    WT = OCOLS // P             # w shard row tiles (4)
    WCH = IN // 2               # w load chunk = one g-half of in features
    NN = 2                      # moving-dim split
    NCH = ROWS // NN            # 512
    assert ROWS % P == 0 and G % P == 0 and OCOLS % P == 0

    core_ids = list(range(ncores))

    nc = bacc.Bacc(num_devices=ncores)

    x_d = nc.declare_dram_parameter("x_loc", [ROWS, IN], F32, isOutput=False)
    w_d = nc.declare_dram_parameter("w_loc", [OCOLS, IN], F32, isOutput=False)
    ws_d = nc.declare_dram_parameter("wscale", [1, 1], F32, isOutput=False)
    bias_d = nc.declare_dram_parameter("bias", [OUT], F32, isOutput=False)
    outT_d = nc.declare_dram_parameter("outT", [OUT, ROWS], F32, isOutput=True)

    # collective bounce buffers
    mx_in = nc.dram_tensor("mx_in", [1, P], BF16)
    mx_out = nc.dram_tensor("mx_out", [ncores, P], BF16, addr_space="Shared")
    g1_in = nc.dram_tensor("g1_in", [GH, OCOLS], FP8)
    g1_out = nc.dram_tensor("g1_out", [ncores * GH, OCOLS], FP8,
                            addr_space="Shared")
    g2_in = nc.dram_tensor("g2_in", [GH, OCOLS], FP8)
    g2_out = nc.dram_tensor("g2_out", [ncores * GH, OCOLS], FP8,
                            addr_space="Shared")

    with tile.TileContext(nc) as tc:
        with (
            tc.tile_pool(name="xa", bufs=8) as xap,       # resident x, recycled as w panels
            tc.tile_pool(name="scr", bufs=3) as scrp,     # w loads / qt / h1 panels
            tc.tile_pool(name="qab", bufs=2) as qabp,
            tc.tile_pool(name="xs", bufs=3) as xsp,
            tc.tile_pool(name="xsT", bufs=1) as xsTp,
            tc.tile_pool(name="wab", bufs=4) as wabp,
            tc.tile_pool(name="wsh", bufs=4) as wshp,
            tc.tile_pool(name="w3T", bufs=2) as w3Tp,
            tc.tile_pool(name="w3T8", bufs=2) as w3T8p,
            tc.tile_pool(name="outp", bufs=3) as outp,
            tc.tile_pool(name="cst", bufs=1) as cst,
            tc.tile_pool(name="ps", bufs=4, space="PSUM") as psp,
        ):
            # ---------------- phase A loads ----------------------------------
            #  sync ring:   w0.wt0 w0.wt1 x0 x2 x4 x6 w1.wt0 w1.wt1
            #  scalar ring: w0.wt2 w0.wt3 [T-h0 + g1 stores] x1 x3 x5 x7
            #               w1.wt2 w1.wt3 [T-h1 + g2 stores]
            wl = {}
            xa = [None] * RT

            def load_w(eng, h, wt):
                t = scrp.tile([P, WCH], F32, tag="scr", name=f"wl{h}_{wt}")
                eng.dma_start(out=t[:], in_=w_d[wt * P:(wt + 1) * P,
                                               h * WCH:(h + 1) * WCH])
                wl[(h, wt)] = t

            def load_x(eng, rt):
                t = xap.tile([P, IN], F32, tag="xa", name=f"xa{rt}")
                eng.dma_start(out=t[:], in_=x_d[rt * P:(rt + 1) * P, :])
                xa[rt] = t

            # w sums on Vector; transpose on scalar (xbar needs bf16),
            # fp8 convert on Vector, store on scalar.
            wsh_t = {}

            def w_sum(h, wt):
                wlh = wl[(h, wt)]
                w3 = wlh[:].rearrange("p (g f) -> p g f", f=4)
                wa = wabp.tile([P, GH], BF16, tag="wab", name=f"wa{h}_{wt}")
                wb = wabp.tile([P, GH], BF16, tag="wab", name=f"wb{h}_{wt}")
                nc.vector.tensor_tensor(out=wa[:], in0=w3[:, :, 0], in1=w3[:, :, 1],
                                        op=mybir.AluOpType.add)
                nc.vector.tensor_tensor(out=wb[:], in0=w3[:, :, 2], in1=w3[:, :, 3],
                                        op=mybir.AluOpType.add)
                wsh = wshp.tile([P, GH], BF16, tag="wsh", name=f"wsh{h}_{wt}")
                nc.vector.tensor_tensor(out=wsh[:], in0=wa[:], in1=wb[:],
                                        op=mybir.AluOpType.add)
                wsh_t[(h, wt)] = wsh

            def w_tx(h, wt, gdst):
                # [128 o, 512 g] -(xbar)-> [128 gp, 4, 128 o] bf16 -> fp8 -> DRAM
                w3T = w3Tp.tile([P, GH // P, P], BF16, tag="w3T",
                                name=f"w3T{h}_{wt}")
                nc.scalar.dma_start_transpose(w3T[:], wsh_t[(h, wt)][:])
                w38 = w3T8p.tile([P, GH // P, P], FP8, tag="w3T8",
                                 name=f"w38{h}_{wt}")
                nc.vector.tensor_copy(out=w38[:], in_=w3T[:])
                nc.scalar.dma_start(
                    out=gdst[0:GH, wt * P:(wt + 1) * P]
                        .rearrange("(a p) o -> p a o", p=P),
                    in_=w38[:])

            load_w(nc.sync, 0, 0)
            load_w(nc.scalar, 0, 2)
            load_w(nc.sync, 0, 1)
            load_w(nc.scalar, 0, 3)
            for wt in (0, 2, 1, 3):
                w_sum(0, wt)
            for wt in range(WT):
                w_tx(0, wt, g1_in)
            nc.gpsimd.collective_compute(
                "AllGather", mybir.AluOpType.bypass,
                replica_groups=[core_ids],
                ins=[g1_in[:]], outs=[g1_out[:]],
            )
            load_x(nc.sync, 0)
            load_x(nc.scalar, 1)
            load_x(nc.sync, 2)
            load_x(nc.scalar, 3)
            load_x(nc.sync, 4)
            load_x(nc.scalar, 5)
            load_x(nc.sync, 6)
            load_x(nc.scalar, 7)
            load_w(nc.sync, 1, 0)
            load_w(nc.scalar, 1, 2)
            load_w(nc.sync, 1, 1)
            load_w(nc.scalar, 1, 3)
            ws_bc = cst.tile([P, 1], F32, tag="ws_bc")
            nc.scalar.dma_start(out=ws_bc[:],
                                in_=bass.AP(ws_d, 0, [[0, P], [1, 1]]))
            bias_sb = cst.tile([P, OBT], F32, tag="bias_sb")
            nc.scalar.dma_start(out=bias_sb[:],
                                in_=bias_d[:].rearrange("(b p) -> p b", p=P))

            # ---------------- absmax -> tiny AllGather -----------------------
            mxcol = cst.tile([P, RT], F32, tag="mxcol")
            for rt in [0, 1, 2, 3, 4, 5, 6, 7]:
                nc.vector.tensor_reduce(out=mxcol[:, rt:rt + 1], in_=xa[rt][:],
                                        axis=mybir.AxisListType.X,
                                        op=mybir.AluOpType.max,
                                        apply_absolute_value=True)
            mx1 = cst.tile([P, 1], F32, tag="mx1")
            nc.vector.tensor_reduce(out=mx1[:], in_=mxcol[:],
                                    axis=mybir.AxisListType.X,
                                    op=mybir.AluOpType.max)
            mxbf = cst.tile([P, 1], BF16, tag="mxbf")
            nc.vector.tensor_copy(out=mxbf[:], in_=mx1[:])
            nc.gpsimd.dma_start(out=mx_in[0:1, :].rearrange("a p -> p a"),
                                in_=mxbf[:])
            nc.gpsimd.collective_compute(
                "AllGather", mybir.AluOpType.bypass,
                replica_groups=[core_ids],
                ins=[mx_in[:]], outs=[mx_out[:]],
            )

            # ---------------- w half-1 sums ----------------------------------
            for wt in (2, 0, 3, 1):
                w_sum(1, wt)
            for wt in range(WT):
                w_tx(1, wt, g2_in)

            # ---------------- global scale (per-partition, no bounce) --------
            mxg = cst.tile([P, ncores * P], BF16, tag="mxg")
            nc.scalar.dma_start(
                out=mxg[:].rearrange("p (c o) -> p c o", c=ncores),
                in_=bass.AP(mx_out, 0, [[0, P], [P, ncores], [1, P]]))
            nc.gpsimd.collective_compute(
                "AllGather", mybir.AluOpType.bypass,
                replica_groups=[core_ids],
                ins=[g2_in[:]], outs=[g2_out[:]],
            )
            mx2 = cst.tile([P, 1], F32, tag="mx2")
            nc.vector.tensor_reduce(out=mx2[:], in_=mxg[:],
                                    axis=mybir.AxisListType.X,
                                    op=mybir.AluOpType.max)
            # recip = 127/max ; asc = max/127 ; sc = ws*asc*0.25
            recip = cst.tile([P, 1], F32, tag="recip")
            nc.vector.reciprocal(out=recip[:], in_=mx2[:])
            nc.vector.tensor_scalar(out=recip[:], in0=recip[:], scalar1=127.0,
                                    scalar2=None, op0=mybir.AluOpType.mult)
            asc = cst.tile([P, 1], F32, tag="asc")
            nc.vector.tensor_scalar(out=asc[:], in0=mx2[:],
                                    scalar1=float(np.float32(1.0 / 127.0)),
                                    scalar2=None, op0=mybir.AluOpType.mult)
            sc_bc = cst.tile([P, 1], F32, tag="sc_bc")
            nc.vector.tensor_tensor(out=sc_bc[:], in0=ws_bc[:], in1=asc[:],
                                    op=mybir.AluOpType.mult)
            nc.vector.tensor_scalar(out=sc_bc[:], in0=sc_bc[:], scalar1=0.25,
                                    scalar2=None, op0=mybir.AluOpType.mult)

            # ---------------- quantize + group-sum + transpose ---------------
            xsT3 = xsTp.tile([P, GT, ROWS], BF16, tag="xsT3")
            qts = {}
            xs_t = {}
            panels = {}

            def ts1(rt):
                eng = nc.vector if rt % 2 == 0 else nc.gpsimd
                eng.tensor_scalar(out=xa[rt][:], in0=xa[rt][:],
                                  scalar1=recip[:], scalar2=MAGIC_C,
                                  op0=mybir.AluOpType.mult,
                                  op1=mybir.AluOpType.add)

            def ext(rt):
                qt = scrp.tile([P, IN], BF16, tag="scr", name=f"qt{rt}")
                nc.scalar.activation(out=qt[:], in_=xa[rt][:],
                                     func=mybir.ActivationFunctionType.Copy,
                                     bias=-MAGIC_C, scale=1.0)
                qts[rt] = qt

            def adds(rt):
                eng = nc.vector if rt % 2 == 0 else nc.gpsimd
                q3 = qts[rt][:].rearrange("p (g f) -> p g f", f=4)
                qa = qabp.tile([P, G], BF16, tag="qab", name=f"qa{rt}")
                qb = qabp.tile([P, G], BF16, tag="qab", name=f"qb{rt}")
                eng.tensor_tensor(out=qa[:], in0=q3[:, :, 0], in1=q3[:, :, 1],
                                  op=mybir.AluOpType.add)
                eng.tensor_tensor(out=qb[:], in0=q3[:, :, 2], in1=q3[:, :, 3],
                                  op=mybir.AluOpType.add)
                xs = xsp.tile([P, G], BF16, tag="xs", name=f"xs{rt}")
                eng.tensor_tensor(out=xs[:], in0=qa[:], in1=qb[:],
                                  op=mybir.AluOpType.add)
                xs_t[rt] = xs

            def tx(rt):
                nc.sync.dma_start_transpose(
                    xsT3[:, :, rt * P:(rt + 1) * P], xs_t[rt][:])

            def panel(c, h):
                # h0 panels recycle the xa slots (gated by ext(c)); h1 panels
                # recycle the scr slots (gated by the late qt adds).
                if h == 0:
                    t = xap.tile([P, GH // P, OCOLS], FP8, tag="xa",
                                 name=f"wst{c}_{h}")
                    eng = nc.scalar if c < 4 else nc.sync
                else:
                    t = scrp.tile([P, GH // P, OCOLS], FP8, tag="scr",
                                  name=f"wst{c}_{h}")
                    eng = nc.sync if c % 2 == 0 else nc.scalar
                src = g1_out if h == 0 else g2_out
                eng.dma_start(
                    out=t[:],
                    in_=src[c * GH:(c + 1) * GH, :]
                        .rearrange("(a p) o -> p a o", p=P))
                panels[(c, h)] = t

            for rt in (0, 2, 4, 6):
                ts1(rt)
            for rt in (1, 3, 5, 7):
                ts1(rt)
            ext(0)
            ext(1)
            panel(0, 0)
            ext(2)
            panel(1, 0)
            ext(3)
            panel(2, 0)
            ext(4)
            panel(3, 0)
            adds(0)
            adds(1)
            tx(0)
            ext(5)
            adds(2)
            adds(3)
            tx(1)
            tx(2)
            ext(6)
            adds(4)
            tx(3)
            panel(4, 0)
            ext(7)
            adds(5)
            adds(6)
            tx(4)
            panel(5, 0)
            tx(5)
            panel(6, 0)
            adds(7)
            tx(6)
            panel(7, 0)
            tx(7)
            for c in range(8):
                panel(c, 1)

            # ---------------- matmul + epilogue ------------------------------
            # per source core c: nn0 k<4, nn0 k>=4, nn1 k<4, nn1 k>=4 so the
            # stream meets each gate (panels h0, AG2 panels, late xsT3
            # columns) in the order they become ready.
            epi_cnt = 0
            for c in range(ncores):
                p1 = panels[(c, 0)]
                p2 = panels[(c, 1)]
                pss = {}
                for oc in range(WT):
                    ob = c * WT + oc
                    pss[oc] = psp.tile([P, ROWS], F32, tag="ps", name=f"ps{ob}")
                for nn in range(NN):
                    for half, ph in ((0, p1), (1, p2)):
                        for oc in range(WT):
                            for kk in range(GT // 2):
                                k = half * 4 + kk
                                nc.tensor.matmul(
                                    pss[oc][:, nn * NCH:(nn + 1) * NCH],
                                    lhsT=ph[:, kk, oc * P:(oc + 1) * P],
                                    rhs=xsT3[:, k, nn * NCH:(nn + 1) * NCH],
                                    start=(k == 0), stop=(k == GT - 1))
                for oc in range(WT):
                    ob = c * WT + oc
                    ot = outp.tile([P, ROWS], F32, tag="outp", name=f"ot{ob}")
                    if epi_cnt % 2 == 0:
                        nc.vector.tensor_scalar(out=ot[:], in0=pss[oc][:],
                                                scalar1=sc_bc[:],
                                                scalar2=bias_sb[:, ob:ob + 1],
                                                op0=mybir.AluOpType.mult,
                                                op1=mybir.AluOpType.add)
                    else:
                        nc.scalar.activation(
                            out=ot[:], in_=pss[oc][:],
                            func=mybir.ActivationFunctionType.Identity,
                            scale=sc_bc[:],
                            bias=bias_sb[:, ob:ob + 1])
                    epi_cnt += 1
                    seng = nc.sync if ob % 2 == 0 else nc.scalar
                    seng.dma_start(
                        out=outT_d[ob * P:(ob + 1) * P, :], in_=ot[:])

    return nc


def make_in_maps(inputs, ncores=NCORES):
    x = np.ascontiguousarray(np.asarray(inputs["input"], dtype=np.float32))
    w = np.ascontiguousarray(np.asarray(inputs["weight"], dtype=np.float32))
    ws = np.asarray(inputs["weight_scale"], dtype=np.float32).reshape(1, 1)
    b = np.ascontiguousarray(np.asarray(inputs["bias"], dtype=np.float32))
    N = x.shape[0]
    OUT = w.shape[0]
    ROWS = N // ncores
    OCOLS = OUT // ncores
    return [
        {
            "x_loc": x[c * ROWS:(c + 1) * ROWS],
            "w_loc": w[c * OCOLS:(c + 1) * OCOLS],
            "wscale": ws,
            "bias": b,
        }
        for c in range(ncores)
    ]


def assemble_output(results):
    return np.ascontiguousarray(
        np.concatenate([np.asarray(r["outT"]).T for r in results], axis=0))


_NC_CACHE = {}


def _get_nc():
    key = (N_FULL, IN_F, OUT_F, NCORES)
    if key not in _NC_CACHE:
        nc = build_bitlinear(*key)
        if not nc.is_finalized():
            nc.finalize()
        _NC_CACHE[key] = nc
    return _NC_CACHE[key]


def run_on_hw(inputs, trace=False):
    from concourse.bass_utils import run_bass_kernel_spmd
    nc = _get_nc()
    in_maps = make_in_maps(inputs)
    res = run_bass_kernel_spmd(nc, in_maps, list(range(NCORES)), trace=trace)
    return assemble_output(res.results), res


def kernel(**inputs) -> np.ndarray:
    out, _ = run_on_hw(inputs, trace=False)
    return out
